# revision 32
# baseline (speedup 1.0000x reference)
"""AnomalyAttention Trainium2 kernel — 8-core SPMD via bass/Tile.

Reference computes, for B=4, L=1024, H=8, E=64:
    scores = einsum('blhe,bshe->bhls', q, k); causal mask; attn = scores/8
    series = softmax(attn, -1)                            [B,H,L,L]
    V      = einsum('bhls,bshd->blhd', series, values)    [B,L,H,E]
    sig    = broadcast(3**(sigmoid(5*sigma)+1e-5) - 1)    [B,H,L,L]
    prior  = 1/sqrt(2pi)/sig * exp(-dist^2/(2*sig^2))     [B,H,L,L]

Sharding: the 32 (b,h) slices are data/head-parallel; core c owns slices
[4c, 4c+4). Each core computes its slices fully independently (no
collectives). Scores are built transposed (s on partitions) so the PV
matmul consumes exp(scores)^T directly; an appended ones-column on the
values operand yields softmax row-sums for free in the same matmul.
Host-side work is layout only: pre-transposed Q/K (so no on-chip
transposes are needed) and re-assembly/transposition of per-core
outputs into the reference layouts.
"""

import math
import os
import sys

import numpy as np

sys.path.insert(0, "/opt/trn_rl_repo")
import ml_dtypes

B, L, H, E = 4, 1024, 8, 64
N_CORES = 8
G_PER_CORE = (B * H) // N_CORES  # 4 slices per core
NCH = L // 128  # 8 chunks of 128 along L
SCALE = 1.0 / math.sqrt(E)
LN3 = math.log(3.0)
INV_SQRT_2PI = 1.0 / math.sqrt(2.0 * math.pi)
# prior is a band matrix: sigma <= 2.0003 makes exp(-d^2/2sig^2) underflow to
# an exact f32 zero for |l-s| >= 29 (reference does the same); compute only a
# 192-wide window centered on the diagonal per 128-row chunk.
PRIOR_W = 192
PRIOR_S0 = [min(max(128 * i - 32, 0), 1024 - PRIOR_W) for i in range(8)]

# float32r = single-pass fp32 matmul mode (TF32-like); 4x faster than
# exact fp32 on the PE. Set ANOM_EXACT_FP32=1 to use exact fp32 matmuls.
USE_F32R = os.environ.get("ANOM_EXACT_FP32", "0") != "1"

_CACHE = {}


def _chunks(l0):
    """Split [l0, L) at the 512-column PSUM bank boundaries."""
    out = []
    c = l0
    for b0 in (0, 512):
        lo, hi = max(c, b0), min(L, b0 + 512)
        if lo < hi:
            out.append((lo, hi - lo))
    return out


def _build():
    import concourse.bacc as bacc
    import concourse.mybir as mybir
    import concourse.tile as tile

    f32 = mybir.dt.float32
    bf16 = mybir.dt.bfloat16
    mm_dt = mybir.dt.float32r if USE_F32R else f32
    Act = mybir.ActivationFunctionType
    AluOp = mybir.AluOpType

    nc = bacc.Bacc("TRN2", target_bir_lowering=False, debug=False,
                   num_devices=N_CORES)

    # Per-core inputs (host pre-packs layouts; see kernel()).
    qT = nc.dram_tensor("qt", [G_PER_CORE, E, L], mm_dt, kind="ExternalInput")
    kT = nc.dram_tensor("kt", [G_PER_CORE, E, L], mm_dt, kind="ExternalInput")
    vp = nc.dram_tensor("vp", [G_PER_CORE, L, E + 1], mybir.dt.bfloat16,
                        kind="ExternalInput")
    sg = nc.dram_tensor("sg", [G_PER_CORE, 128, NCH], f32, kind="ExternalInput")
    d2 = nc.dram_tensor("d2", [128, NCH * PRIOR_W], f32, kind="ExternalInput")
    mka = nc.dram_tensor("mka", [128, 128], mybir.dt.bfloat16,
                         kind="ExternalInput")

    # Per-core outputs.
    st_o = nc.dram_tensor("st", [G_PER_CORE, L, L], bf16, kind="ExternalOutput")
    pr_o = nc.dram_tensor("pr", [G_PER_CORE, L, L], bf16, kind="ExternalOutput")
    vt_o = nc.dram_tensor("vt", [G_PER_CORE, E, L], bf16,
                          kind="ExternalOutput")
    sig_o = nc.dram_tensor("sig", [G_PER_CORE, 128, NCH], f32,
                           kind="ExternalOutput")

    qT, kT, vp, sg, d2, mka = (t.ap() for t in (qT, kT, vp, sg, d2, mka))
    st_o, pr_o, vt_o, sig_o = (t.ap() for t in (st_o, pr_o, vt_o, sig_o))

    with tile.TileContext(nc) as tc:
        with (
            tc.tile_pool(name="const", bufs=1) as constp,
            tc.tile_pool(name="io", bufs=2) as iop,
            tc.tile_pool(name="sigp", bufs=1) as sigp,
            tc.tile_pool(name="et", bufs=4) as etp,
            tc.tile_pool(name="sm", bufs=2) as smp,
            tc.tile_pool(name="st", bufs=4) as stp,
            tc.tile_pool(name="pri", bufs=6) as prip,
            tc.tile_pool(name="ps_s", bufs=2, space="PSUM") as ps_s,
            tc.tile_pool(name="ps_v", bufs=2, space="PSUM") as ps_v,
        ):
            d2_t = constp.tile([128, NCH * PRIOR_W], f32, tag="d2")
            mka_t = constp.tile([128, 128], bf16, tag="mka")
            nc.sync.dma_start(mka_t[:], mka)
            b3_t = constp.tile([128, 1], f32, tag="b3")
            nc.vector.memset(b3_t[:], LN3 * 1e-5)

            # ---- sigma transform for all slices up-front (batches the
            # Exp/Ln activations so the ACT table set switches at most a
            # couple of times before the main all-Exp phase). ----
            sig_coef = []
            for g in range(G_PER_CORE):
                srw = sigp.tile([128, NCH], f32, tag=f"sraw{g}")
                nc.scalar.dma_start(srw[:], sg[g])
                u = sigp.tile([128, NCH], f32, tag=f"u{g}")
                nc.scalar.activation(u[:], srw[:], Act.Exp, scale=-5.0)
                w = sigp.tile([128, NCH], f32, tag=f"w{g}")
                nc.vector.tensor_scalar_add(w[:], u[:], 1.0)
                s0 = sigp.tile([128, NCH], f32, tag=f"s0{g}")
                nc.vector.reciprocal(s0[:], w[:])  # sigmoid(5x)
                sigv = sigp.tile([128, NCH], f32, tag=f"sv{g}")
                nc.scalar.activation(sigv[:], s0[:], Act.Exp, scale=LN3,
                                     bias=b3_t[:])  # 3**(s0+1e-5)
                nc.vector.tensor_scalar_add(sigv[:], sigv[:], -1.0)
                s2 = sigp.tile([128, NCH], f32, tag=f"s2{g}")
                nc.vector.tensor_mul(s2[:], sigv[:], sigv[:])
                r2 = sigp.tile([128, NCH], f32, tag=f"r2{g}")
                nc.vector.reciprocal(r2[:], s2[:])
                a_t = sigp.tile([128, NCH], f32, tag=f"a{g}")
                nc.vector.tensor_scalar_mul(a_t[:], r2[:], -0.5)
                sig_coef.append((sigv, a_t))
            lb_ts = []
            for g in range(G_PER_CORE):
                sigv, _ = sig_coef[g]
                lb = sigp.tile([128, NCH], f32, tag=f"lb{g}")
                # ln(sig/c) then negate -> ln(c/sig)
                nc.scalar.activation(lb[:], sigv[:], Act.Ln,
                                     scale=1.0 / INV_SQRT_2PI)
                nc.vector.tensor_scalar_mul(lb[:], lb[:], -1.0)
                lb_ts.append(lb)

            def emit_prior(g, i):
                _, a_t = sig_coef[g]
                lb_t = lb_ts[g]
                pt = prip.tile([128, PRIOR_W], bf16, tag="pt")
                nc.scalar.activation(
                    pt[:], d2_t[:, i * PRIOR_W:(i + 1) * PRIOR_W],
                    Act.Exp, scale=a_t[:, i:i + 1], bias=lb_t[:, i:i + 1])
                eng = nc.sync if i % 2 == 0 else nc.gpsimd
                eng.dma_start(
                    pr_o[g, i * 128:(i + 1) * 128,
                         PRIOR_S0[i]:PRIOR_S0[i] + PRIOR_W], pt[:])

            def load_inputs(g):
                qt_t = iop.tile([E, L], mm_dt, tag="qt")
                nc.sync.dma_start(qt_t[:], qT[g])
                kt_t = iop.tile([E, L], mm_dt, tag="kt")
                nc.sync.dma_start(kt_t[:], kT[g])
                vp_t = iop.tile([128, NCH * (E + 1)], bf16, tag="vp")
                nc.sync.dma_start(
                    vp_t[:].rearrange("p (c w) -> p c w", w=E + 1),
                    vp[g].rearrange("(c p) w -> p c w", p=128),
                )
                return qt_t, kt_t, vp_t

            nxt_inputs = load_inputs(0)
            for g in range(G_PER_CORE):
                qt_t, kt_t, vp_t = nxt_inputs

                # vt accumulator [E+1, L]: row E collects softmax row-sums.
                vtp = ps_v.tile([E + 1, L], f32, tag="vt")

                # Group chunks into <=1024-wide (2-bank) PSUM tiles so each
                # group needs a single exp activation while leaving room to
                # double-buffer the vt accumulator.
                GROUPS = [(0,), (1, 7), (2, 6), (3, 5), (4,)]
                ets = {}
                pr_q = list(range(NCH))  # prev slice's prior chunks to emit
                for gi, grp in enumerate(GROUPS):
                    members = []
                    off = 0
                    for j in grp:
                        members.append((j, 128 * j, off, L - 128 * j))
                        off += L - 128 * j
                    width = off
                    et = etp.tile([128, width], bf16, tag=f"et{gi}")
                    ps = ps_s.tile([128, width], f32, tag="ps")
                    if g == 0:
                        # just-in-time d2 chunks; a single up-front 4MB DMA
                        # stalls the whole pipeline start for ~14us.
                        for i in range(gi * 2, min(gi * 2 + 2, NCH)):
                            nc.gpsimd.dma_start(
                                d2_t[:, i * PRIOR_W:(i + 1) * PRIOR_W],
                                d2[:, i * PRIOR_W:(i + 1) * PRIOR_W])
                    for (j, l0, off, w) in members:
                        r0 = off
                        while r0 < off + w:
                            rw = min(512 - r0 % 512, off + w - r0)
                            nc.tensor.matmul(
                                ps[:, r0:r0 + rw],
                                kt_t[:, l0:l0 + 128],
                                qt_t[:, l0 + (r0 - off):l0 + (r0 - off) + rw],
                                start=True, stop=True,
                            )
                            r0 += rw
                    nc.scalar.activation(et[:], ps[:], Act.Exp, scale=SCALE)
                    for (j, l0, off, w) in members:
                        # causal mask on the diag block (keep s <= l)
                        nc.vector.tensor_mul(et[:, off:off + 128],
                                             et[:, off:off + 128], mka_t[:])
                    if g > 0:
                        for _ in range(2):
                            if pr_q:
                                emit_prior(g - 1, pr_q.pop(0))
                    for (j, l0, off, w) in members:
                        for (c0, cw) in _chunks(l0):
                            nc.tensor.matmul(
                                vtp[:, c0:c0 + cw],
                                vp_t[:, j * (E + 1):(j + 1) * (E + 1)],
                                et[:, off + c0 - l0:off + c0 - l0 + cw],
                                start=(j == 0), stop=(j == NCH - 1),
                                skip_group_check=True,
                            )
                        ets[j] = (et, off)

                if g + 1 < G_PER_CORE:
                    nxt_inputs = load_inputs(g + 1)

                # 1/rowsum (fast approx is ~3e-6 rel err, plenty here),
                # then broadcast down all 128 partitions on the idle GpSimd.
                rs = smp.tile([1, L], f32, tag="rs")
                nc.scalar.copy(rs[:], vtp[E:E + 1, :])
                rr = smp.tile([1, L], f32, tag="rr")
                nc.vector.reciprocal_approx_fast(rr[:], rs[:])
                rr_b = smp.tile([1, L], bf16, tag="rr_b")
                nc.vector.tensor_copy(rr_b[:], rr[:])
                bc_b = smp.tile([128, L], bf16, tag="bc_b")
                nc.gpsimd.partition_broadcast(bc_b[:], rr_b[:])

                # normalize V^T in bf16 and store (host upcasts)
                vtb = smp.tile([E, L], bf16, tag="vtb")
                nc.vector.tensor_copy(vtb[:], vtp[:E, :])
                vn = smp.tile([E, L], bf16, tag="vn")
                nc.vector.tensor_mul(vn[:], vtb[:], bc_b[:E, :])
                nc.sync.dma_start(vt_o[g], vn[:])

                # normalize series^T in place and store; the strictly
                # lower-triangular remainder of st_o stays pre-zeroed.
                for j in range(NCH):
                    l0 = 128 * j
                    et, off = ets[j]
                    st_t = stp.tile([128, L - l0], bf16, tag=f"st{j}")
                    nc.vector.tensor_mul(
                        st_t[:], et[:, off:off + L - l0], bc_b[:, l0:])
                    nc.sync.dma_start(st_o[g, l0:l0 + 128, l0:], st_t[:])

            # final slice's priors overlap the last epilogue
            for i in range(NCH):
                emit_prior(G_PER_CORE - 1, i)

            # tiny sigma outputs last so they never gate the input queue
            for g in range(G_PER_CORE):
                nc.sync.dma_start(sig_o[g], sig_coef[g][0][:])

    nc.compile()
    return nc


def _host_inputs(queries, keys, values, sigma):
    q = np.ascontiguousarray(
        queries.transpose(0, 2, 3, 1)).reshape(B * H, E, L)
    k = np.ascontiguousarray(keys.transpose(0, 2, 3, 1)).reshape(B * H, E, L)
    v = np.ascontiguousarray(
        values.transpose(0, 2, 1, 3)).reshape(B * H, L, E)
    vp = np.concatenate([v, np.ones((B * H, L, 1), np.float32)],
                        axis=2).astype(ml_dtypes.bfloat16)
    sgt = np.ascontiguousarray(
        sigma.transpose(0, 2, 1).reshape(B * H, NCH, 128).transpose(0, 2, 1))
    p = np.arange(128, dtype=np.float32)
    w = np.arange(PRIOR_W, dtype=np.float32)
    d2 = np.empty((128, NCH, PRIOR_W), np.float32)
    for i in range(NCH):
        d2[:, i, :] = (128 * i + p[:, None] - (PRIOR_S0[i] + w[None, :])) ** 2
    d2 = np.ascontiguousarray(d2.reshape(128, NCH * PRIOR_W))
    mka = np.where(np.arange(128)[:, None] <= np.arange(128)[None, :],
                   1.0, 0.0).astype(ml_dtypes.bfloat16)
    in_maps = []
    for c in range(N_CORES):
        sl = slice(G_PER_CORE * c, G_PER_CORE * (c + 1))
        in_maps.append({
            "qt": np.ascontiguousarray(q[sl]),
            "kt": np.ascontiguousarray(k[sl]),
            "vp": np.ascontiguousarray(vp[sl]),
            "sg": np.ascontiguousarray(sgt[sl]),
            "d2": d2, "mka": mka,
        })
    return in_maps


LAST_EXEC_NS = None


def kernel(queries, keys, values, sigma):
    global LAST_EXEC_NS
    import concourse.bass_utils as bass_utils

    queries = np.asarray(queries, dtype=np.float32)
    keys = np.asarray(keys, dtype=np.float32)
    values = np.asarray(values, dtype=np.float32)
    sigma = np.asarray(sigma, dtype=np.float32)

    if "nc" not in _CACHE:
        _CACHE["nc"] = _build()
    nc = _CACHE["nc"]
    in_maps = _host_inputs(queries, keys, values, sigma)

    trace = os.environ.get("ANOM_TRACE", "0") == "1"
    kwargs = {}
    if trace:
        import contextlib
        import ctypes
        import types

        if "antenv.axon_hooks" not in sys.modules:
            boot = "/root/.axon_site/trn_agent_boot"
            if boot not in sys.path:
                sys.path.insert(0, boot)
            import trn_boot
            hook = trn_boot._ntff_profile_via_ctypes(
                "/opt/axon/libaxon_pjrt.so")
            mod = types.ModuleType("antenv.axon_hooks")
            mod.get_axon_ntff_profile_hook = lambda: hook
            mod.set_axon_ntff_profile_hook = lambda h: None
            sys.modules["antenv.axon_hooks"] = mod
        bass_utils.upload_artifacts = lambda tmpdir: f"file://{tmpdir}"
        kwargs["trace"] = True

    res = bass_utils.run_bass_kernel_spmd(
        nc, in_maps, core_ids=list(range(N_CORES)), **kwargs)
    LAST_EXEC_NS = res.exec_time_ns

    V = np.empty((B, L, H, E), np.float32)
    series = np.empty((B, H, L, L), np.float32)
    prior = np.empty((B, H, L, L), np.float32)
    sig_small = np.empty((B, H, L), np.float32)
    for c in range(N_CORES):
        r = res.results[c]
        for li in range(G_PER_CORE):
            g = G_PER_CORE * c + li
            b, h = g // H, g % H
            series[b, h] = r["st"][li].T.astype(np.float32)
            prior[b, h] = r["pr"][li].astype(np.float32)
            V[b, :, h, :] = r["vt"][li].T.astype(np.float32)
            sig_small[b, h] = r["sig"][li].T.reshape(L)
    sig = np.broadcast_to(sig_small[..., None], (B, H, L, L))
    return V, series, prior, sig


# revision 33
# speedup vs baseline: 1.0728x; 1.0728x over previous
"""AnomalyAttention Trainium2 kernel — 8-core SPMD via bass/Tile.

Reference computes, for B=4, L=1024, H=8, E=64:
    scores = einsum('blhe,bshe->bhls', q, k); causal mask; attn = scores/8
    series = softmax(attn, -1)                            [B,H,L,L]
    V      = einsum('bhls,bshd->blhd', series, values)    [B,L,H,E]
    sig    = broadcast(3**(sigmoid(5*sigma)+1e-5) - 1)    [B,H,L,L]
    prior  = 1/sqrt(2pi)/sig * exp(-dist^2/(2*sig^2))     [B,H,L,L]

Sharding: the 32 (b,h) slices are data/head-parallel; core c owns slices
[4c, 4c+4). Each core computes its slices fully independently (no
collectives). Scores are built transposed (s on partitions) so the PV
matmul consumes exp(scores)^T directly; an appended ones-column on the
values operand yields softmax row-sums for free in the same matmul.
Host-side work is layout only: pre-transposed Q/K (so no on-chip
transposes are needed) and re-assembly/transposition of per-core
outputs into the reference layouts.
"""

import math
import os
import sys

import numpy as np

sys.path.insert(0, "/opt/trn_rl_repo")
import ml_dtypes

B, L, H, E = 4, 1024, 8, 64
N_CORES = 8
G_PER_CORE = (B * H) // N_CORES  # 4 slices per core
NCH = L // 128  # 8 chunks of 128 along L
SCALE = 1.0 / math.sqrt(E)
LN3 = math.log(3.0)
INV_SQRT_2PI = 1.0 / math.sqrt(2.0 * math.pi)
# prior is a band matrix: sigma <= 2.0003 makes exp(-d^2/2sig^2) underflow to
# an exact f32 zero for |l-s| >= 29 (reference does the same); compute only a
# 192-wide window centered on the diagonal per 128-row chunk.
PRIOR_W = 192
PRIOR_S0 = [min(max(128 * i - 32, 0), 1024 - PRIOR_W) for i in range(8)]

# float32r = single-pass fp32 matmul mode (TF32-like); 4x faster than
# exact fp32 on the PE. Set ANOM_EXACT_FP32=1 to use exact fp32 matmuls.
USE_F32R = os.environ.get("ANOM_EXACT_FP32", "0") != "1"

_CACHE = {}


def _chunks(l0):
    """Split [l0, L) at the 512-column PSUM bank boundaries."""
    out = []
    c = l0
    for b0 in (0, 512):
        lo, hi = max(c, b0), min(L, b0 + 512)
        if lo < hi:
            out.append((lo, hi - lo))
    return out


def _build():
    import concourse.bacc as bacc
    import concourse.mybir as mybir
    import concourse.tile as tile

    f32 = mybir.dt.float32
    bf16 = mybir.dt.bfloat16
    mm_dt = mybir.dt.float32r if USE_F32R else f32
    Act = mybir.ActivationFunctionType
    AluOp = mybir.AluOpType

    nc = bacc.Bacc("TRN2", target_bir_lowering=False, debug=False,
                   num_devices=N_CORES)

    # Per-core inputs (host pre-packs layouts; see kernel()).
    qT = nc.dram_tensor("qt", [G_PER_CORE, E, L], mm_dt, kind="ExternalInput")
    kT = nc.dram_tensor("kt", [G_PER_CORE, E, L], mm_dt, kind="ExternalInput")
    vp = nc.dram_tensor("vp", [G_PER_CORE, L, E + 1], mybir.dt.bfloat16,
                        kind="ExternalInput")
    sg = nc.dram_tensor("sg", [G_PER_CORE, 128, NCH], f32, kind="ExternalInput")
    d2 = nc.dram_tensor("d2", [128, NCH * PRIOR_W], f32, kind="ExternalInput")
    mka = nc.dram_tensor("mka", [128, 128], mybir.dt.bfloat16,
                         kind="ExternalInput")

    # Per-core outputs.
    st_o = nc.dram_tensor("st", [G_PER_CORE, L, L], bf16, kind="ExternalOutput")
    pr_o = nc.dram_tensor("pr", [G_PER_CORE, L, L], bf16, kind="ExternalOutput")
    vt_o = nc.dram_tensor("vt", [G_PER_CORE, E, L], f32, kind="ExternalOutput")
    sig_o = nc.dram_tensor("sig", [G_PER_CORE, 128, NCH], f32,
                           kind="ExternalOutput")

    qT, kT, vp, sg, d2, mka = (t.ap() for t in (qT, kT, vp, sg, d2, mka))
    st_o, pr_o, vt_o, sig_o = (t.ap() for t in (st_o, pr_o, vt_o, sig_o))

    with tile.TileContext(nc) as tc:
        with (
            tc.tile_pool(name="const", bufs=1) as constp,
            tc.tile_pool(name="io", bufs=2) as iop,
            tc.tile_pool(name="sigp", bufs=1) as sigp,
            tc.tile_pool(name="et", bufs=4) as etp,
            tc.tile_pool(name="sm", bufs=2) as smp,
            tc.tile_pool(name="st", bufs=4) as stp,
            tc.tile_pool(name="pri", bufs=6) as prip,
            tc.tile_pool(name="ps_s", bufs=2, space="PSUM") as ps_s,
            tc.tile_pool(name="ps_v", bufs=2, space="PSUM") as ps_v,
        ):
            d2_t = constp.tile([128, NCH * PRIOR_W], f32, tag="d2")
            mka_t = constp.tile([128, 128], bf16, tag="mka")
            nc.sync.dma_start(mka_t[:], mka)
            b3_t = constp.tile([128, 1], f32, tag="b3")
            nc.vector.memset(b3_t[:], LN3 * 1e-5)

            # ---- sigma transform for all slices up-front (batches the
            # Exp/Ln activations so the ACT table set switches at most a
            # couple of times before the main all-Exp phase). ----
            sig_coef = []
            for g in range(G_PER_CORE):
                srw = sigp.tile([128, NCH], f32, tag=f"sraw{g}")
                nc.scalar.dma_start(srw[:], sg[g])
                u = sigp.tile([128, NCH], f32, tag=f"u{g}")
                nc.scalar.activation(u[:], srw[:], Act.Exp, scale=-5.0)
                w = sigp.tile([128, NCH], f32, tag=f"w{g}")
                nc.vector.tensor_scalar_add(w[:], u[:], 1.0)
                s0 = sigp.tile([128, NCH], f32, tag=f"s0{g}")
                nc.vector.reciprocal(s0[:], w[:])  # sigmoid(5x)
                sigv = sigp.tile([128, NCH], f32, tag=f"sv{g}")
                nc.scalar.activation(sigv[:], s0[:], Act.Exp, scale=LN3,
                                     bias=b3_t[:])  # 3**(s0+1e-5)
                nc.vector.tensor_scalar_add(sigv[:], sigv[:], -1.0)
                s2 = sigp.tile([128, NCH], f32, tag=f"s2{g}")
                nc.vector.tensor_mul(s2[:], sigv[:], sigv[:])
                r2 = sigp.tile([128, NCH], f32, tag=f"r2{g}")
                nc.vector.reciprocal(r2[:], s2[:])
                a_t = sigp.tile([128, NCH], f32, tag=f"a{g}")
                nc.vector.tensor_scalar_mul(a_t[:], r2[:], -0.5)
                sig_coef.append((sigv, a_t))
            lb_ts = []
            for g in range(G_PER_CORE):
                sigv, _ = sig_coef[g]
                lb = sigp.tile([128, NCH], f32, tag=f"lb{g}")
                # ln(sig/c) then negate -> ln(c/sig)
                nc.scalar.activation(lb[:], sigv[:], Act.Ln,
                                     scale=1.0 / INV_SQRT_2PI)
                nc.vector.tensor_scalar_mul(lb[:], lb[:], -1.0)
                lb_ts.append(lb)

            def emit_prior(g, i):
                _, a_t = sig_coef[g]
                lb_t = lb_ts[g]
                pt = prip.tile([128, PRIOR_W], bf16, tag="pt")
                nc.scalar.activation(
                    pt[:], d2_t[:, i * PRIOR_W:(i + 1) * PRIOR_W],
                    Act.Exp, scale=a_t[:, i:i + 1], bias=lb_t[:, i:i + 1])
                eng = nc.sync if i % 2 == 0 else nc.gpsimd
                eng.dma_start(
                    pr_o[g, i * 128:(i + 1) * 128,
                         PRIOR_S0[i]:PRIOR_S0[i] + PRIOR_W], pt[:])

            def load_inputs(g):
                qt_t = iop.tile([E, L], mm_dt, tag="qt")
                nc.sync.dma_start(qt_t[:], qT[g])
                kt_t = iop.tile([E, L], mm_dt, tag="kt")
                nc.sync.dma_start(kt_t[:], kT[g])
                vp_t = iop.tile([128, NCH * (E + 1)], bf16, tag="vp")
                nc.sync.dma_start(
                    vp_t[:].rearrange("p (c w) -> p c w", w=E + 1),
                    vp[g].rearrange("(c p) w -> p c w", p=128),
                )
                return qt_t, kt_t, vp_t

            nxt_inputs = load_inputs(0)
            for g in range(G_PER_CORE):
                qt_t, kt_t, vp_t = nxt_inputs

                # vt accumulator [E+1, L]: row E collects softmax row-sums.
                vtp = ps_v.tile([E + 1, L], f32, tag="vt")

                # Group chunks into <=1024-wide (2-bank) PSUM tiles so each
                # group needs a single exp activation while leaving room to
                # double-buffer the vt accumulator.
                GROUPS = [(0,), (1, 7), (2, 6), (3, 5), (4,)]
                ets = {}
                pr_q = list(range(NCH))  # prev slice's prior chunks to emit
                for gi, grp in enumerate(GROUPS):
                    members = []
                    off = 0
                    for j in grp:
                        members.append((j, 128 * j, off, L - 128 * j))
                        off += L - 128 * j
                    width = off
                    et = etp.tile([128, width], bf16, tag=f"et{gi}")
                    ps = ps_s.tile([128, width], f32, tag="ps")
                    if g == 0:
                        # just-in-time d2 chunks; a single up-front 4MB DMA
                        # stalls the whole pipeline start for ~14us.
                        for i in range(gi * 2, min(gi * 2 + 2, NCH)):
                            nc.gpsimd.dma_start(
                                d2_t[:, i * PRIOR_W:(i + 1) * PRIOR_W],
                                d2[:, i * PRIOR_W:(i + 1) * PRIOR_W])
                    for (j, l0, off, w) in members:
                        r0 = off
                        while r0 < off + w:
                            rw = min(512 - r0 % 512, off + w - r0)
                            nc.tensor.matmul(
                                ps[:, r0:r0 + rw],
                                kt_t[:, l0:l0 + 128],
                                qt_t[:, l0 + (r0 - off):l0 + (r0 - off) + rw],
                                start=True, stop=True,
                            )
                            r0 += rw
                    nc.scalar.activation(et[:], ps[:], Act.Exp, scale=SCALE)
                    for (j, l0, off, w) in members:
                        # causal mask on the diag block (keep s <= l)
                        nc.vector.tensor_mul(et[:, off:off + 128],
                                             et[:, off:off + 128], mka_t[:])
                    if g > 0:
                        for _ in range(2):
                            if pr_q:
                                emit_prior(g - 1, pr_q.pop(0))
                    for (j, l0, off, w) in members:
                        for (c0, cw) in _chunks(l0):
                            nc.tensor.matmul(
                                vtp[:, c0:c0 + cw],
                                vp_t[:, j * (E + 1):(j + 1) * (E + 1)],
                                et[:, off + c0 - l0:off + c0 - l0 + cw],
                                start=(j == 0), stop=(j == NCH - 1),
                                skip_group_check=True,
                            )
                        ets[j] = (et, off)

                if g + 1 < G_PER_CORE:
                    nxt_inputs = load_inputs(g + 1)

                # 1/rowsum (fast approx is ~3e-6 rel err, plenty here),
                # then broadcast down all 128 partitions on the idle GpSimd.
                rs = smp.tile([1, L], f32, tag="rs")
                nc.scalar.copy(rs[:], vtp[E:E + 1, :])
                rr = smp.tile([1, L], f32, tag="rr")
                nc.vector.reciprocal_approx_fast(rr[:], rs[:])
                bc_s = smp.tile([128, L], f32, tag="bc_s")
                nc.gpsimd.partition_broadcast(bc_s[:], rr[:])
                bc_b = smp.tile([128, L], bf16, tag="bc_b")
                nc.vector.tensor_copy(bc_b[:], bc_s[:])

                # normalize V^T and store
                vn = smp.tile([E, L], f32, tag="vn")
                nc.vector.tensor_mul(vn[:], vtp[:E, :], bc_s[:E, :])
                nc.sync.dma_start(vt_o[g], vn[:])

                # normalize series^T in place and store; the strictly
                # lower-triangular remainder of st_o stays pre-zeroed.
                for j in range(NCH):
                    l0 = 128 * j
                    et, off = ets[j]
                    st_t = stp.tile([128, L - l0], bf16, tag=f"st{j}")
                    nc.vector.tensor_mul(
                        st_t[:], et[:, off:off + L - l0], bc_b[:, l0:])
                    nc.sync.dma_start(st_o[g, l0:l0 + 128, l0:], st_t[:])

            # final slice's priors overlap the last epilogue
            for i in range(NCH):
                emit_prior(G_PER_CORE - 1, i)

            # tiny sigma outputs last so they never gate the input queue
            for g in range(G_PER_CORE):
                nc.sync.dma_start(sig_o[g], sig_coef[g][0][:])

    nc.compile()
    return nc


def _host_inputs(queries, keys, values, sigma):
    q = np.ascontiguousarray(
        queries.transpose(0, 2, 3, 1)).reshape(B * H, E, L)
    k = np.ascontiguousarray(keys.transpose(0, 2, 3, 1)).reshape(B * H, E, L)
    v = np.ascontiguousarray(
        values.transpose(0, 2, 1, 3)).reshape(B * H, L, E)
    vp = np.concatenate([v, np.ones((B * H, L, 1), np.float32)],
                        axis=2).astype(ml_dtypes.bfloat16)
    sgt = np.ascontiguousarray(
        sigma.transpose(0, 2, 1).reshape(B * H, NCH, 128).transpose(0, 2, 1))
    p = np.arange(128, dtype=np.float32)
    w = np.arange(PRIOR_W, dtype=np.float32)
    d2 = np.empty((128, NCH, PRIOR_W), np.float32)
    for i in range(NCH):
        d2[:, i, :] = (128 * i + p[:, None] - (PRIOR_S0[i] + w[None, :])) ** 2
    d2 = np.ascontiguousarray(d2.reshape(128, NCH * PRIOR_W))
    mka = np.where(np.arange(128)[:, None] <= np.arange(128)[None, :],
                   1.0, 0.0).astype(ml_dtypes.bfloat16)
    in_maps = []
    for c in range(N_CORES):
        sl = slice(G_PER_CORE * c, G_PER_CORE * (c + 1))
        in_maps.append({
            "qt": np.ascontiguousarray(q[sl]),
            "kt": np.ascontiguousarray(k[sl]),
            "vp": np.ascontiguousarray(vp[sl]),
            "sg": np.ascontiguousarray(sgt[sl]),
            "d2": d2, "mka": mka,
        })
    return in_maps


LAST_EXEC_NS = None


def kernel(queries, keys, values, sigma):
    global LAST_EXEC_NS
    import concourse.bass_utils as bass_utils

    queries = np.asarray(queries, dtype=np.float32)
    keys = np.asarray(keys, dtype=np.float32)
    values = np.asarray(values, dtype=np.float32)
    sigma = np.asarray(sigma, dtype=np.float32)

    if "nc" not in _CACHE:
        _CACHE["nc"] = _build()
    nc = _CACHE["nc"]
    in_maps = _host_inputs(queries, keys, values, sigma)

    trace = os.environ.get("ANOM_TRACE", "0") == "1"
    kwargs = {}
    if trace:
        import contextlib
        import ctypes
        import types

        if "antenv.axon_hooks" not in sys.modules:
            boot = "/root/.axon_site/trn_agent_boot"
            if boot not in sys.path:
                sys.path.insert(0, boot)
            import trn_boot
            hook = trn_boot._ntff_profile_via_ctypes(
                "/opt/axon/libaxon_pjrt.so")
            mod = types.ModuleType("antenv.axon_hooks")
            mod.get_axon_ntff_profile_hook = lambda: hook
            mod.set_axon_ntff_profile_hook = lambda h: None
            sys.modules["antenv.axon_hooks"] = mod
        bass_utils.upload_artifacts = lambda tmpdir: f"file://{tmpdir}"
        kwargs["trace"] = True

    res = bass_utils.run_bass_kernel_spmd(
        nc, in_maps, core_ids=list(range(N_CORES)), **kwargs)
    LAST_EXEC_NS = res.exec_time_ns

    V = np.empty((B, L, H, E), np.float32)
    series = np.empty((B, H, L, L), np.float32)
    prior = np.empty((B, H, L, L), np.float32)
    sig_small = np.empty((B, H, L), np.float32)
    for c in range(N_CORES):
        r = res.results[c]
        for li in range(G_PER_CORE):
            g = G_PER_CORE * c + li
            b, h = g // H, g % H
            series[b, h] = r["st"][li].T.astype(np.float32)
            prior[b, h] = r["pr"][li].astype(np.float32)
            V[b, :, h, :] = r["vt"][li].T
            sig_small[b, h] = r["sig"][li].T.reshape(L)
    sig = np.broadcast_to(sig_small[..., None], (B, H, L, L))
    return V, series, prior, sig


# revision 34
# speedup vs baseline: 1.2871x; 1.1998x over previous
"""AnomalyAttention Trainium2 kernel — 8-core SPMD via bass/Tile.

Reference computes, for B=4, L=1024, H=8, E=64:
    scores = einsum('blhe,bshe->bhls', q, k); causal mask; attn = scores/8
    series = softmax(attn, -1)                            [B,H,L,L]
    V      = einsum('bhls,bshd->blhd', series, values)    [B,L,H,E]
    sig    = broadcast(3**(sigmoid(5*sigma)+1e-5) - 1)    [B,H,L,L]
    prior  = 1/sqrt(2pi)/sig * exp(-dist^2/(2*sig^2))     [B,H,L,L]

Sharding: the 32 (b,h) slices are data/head-parallel; core c owns slices
[4c, 4c+4). Each core computes its slices fully independently (no
collectives). Scores are built transposed (s on partitions) so the PV
matmul consumes exp(scores)^T directly; an appended ones-column on the
values operand yields softmax row-sums for free in the same matmul.
Host-side work is layout only: pre-transposed Q/K (so no on-chip
transposes are needed) and re-assembly/transposition of per-core
outputs into the reference layouts.
"""

import math
import os
import sys

import numpy as np

sys.path.insert(0, "/opt/trn_rl_repo")
import ml_dtypes

B, L, H, E = 4, 1024, 8, 64
N_CORES = 8
G_PER_CORE = (B * H) // N_CORES  # 4 slices per core
NCH = L // 128  # 8 chunks of 128 along L
SCALE = 1.0 / math.sqrt(E)
LN3 = math.log(3.0)
INV_SQRT_2PI = 1.0 / math.sqrt(2.0 * math.pi)
# prior is a band matrix: sigma <= 2.0003 makes exp(-d^2/2sig^2) underflow to
# an exact f32 zero for |l-s| >= 29 (reference does the same); compute only a
# 192-wide window centered on the diagonal per 128-row chunk.
PRIOR_W = 192
PRIOR_S0 = [min(max(128 * i - 32, 0), 1024 - PRIOR_W) for i in range(8)]

# float32r = single-pass fp32 matmul mode (TF32-like); 4x faster than
# exact fp32 on the PE. Set ANOM_EXACT_FP32=1 to use exact fp32 matmuls.
USE_F32R = os.environ.get("ANOM_EXACT_FP32", "0") != "1"

_CACHE = {}


def _chunks(l0):
    """Split [l0, L) at the 512-column PSUM bank boundaries."""
    out = []
    c = l0
    for b0 in (0, 512):
        lo, hi = max(c, b0), min(L, b0 + 512)
        if lo < hi:
            out.append((lo, hi - lo))
    return out


def _build():
    import concourse.bacc as bacc
    import concourse.mybir as mybir
    import concourse.tile as tile

    f32 = mybir.dt.float32
    bf16 = mybir.dt.bfloat16
    mm_dt = mybir.dt.float32r if USE_F32R else f32
    Act = mybir.ActivationFunctionType
    AluOp = mybir.AluOpType

    nc = bacc.Bacc("TRN2", target_bir_lowering=False, debug=False,
                   num_devices=N_CORES)

    # Per-core inputs (host pre-packs layouts; see kernel()).
    qT = nc.dram_tensor("qt", [G_PER_CORE, E, L], mm_dt, kind="ExternalInput")
    kT = nc.dram_tensor("kt", [G_PER_CORE, E, L], mm_dt, kind="ExternalInput")
    vp = nc.dram_tensor("vp", [G_PER_CORE, L, E + 1], mybir.dt.bfloat16,
                        kind="ExternalInput")
    sg = nc.dram_tensor("sg", [G_PER_CORE, 128, NCH], f32, kind="ExternalInput")
    d2 = nc.dram_tensor("d2", [128, NCH * PRIOR_W], f32, kind="ExternalInput")
    mka = nc.dram_tensor("mka", [128, 128], mybir.dt.bfloat16,
                         kind="ExternalInput")

    # Per-core outputs.
    st_o = nc.dram_tensor("st", [G_PER_CORE, L, L], bf16, kind="ExternalOutput")
    pr_o = nc.dram_tensor("pr", [G_PER_CORE, L, L], bf16, kind="ExternalOutput")
    vt_o = nc.dram_tensor("vt", [G_PER_CORE, E, L], f32, kind="ExternalOutput")
    sig_o = nc.dram_tensor("sig", [G_PER_CORE, 128, NCH], f32,
                           kind="ExternalOutput")

    qT, kT, vp, sg, d2, mka = (t.ap() for t in (qT, kT, vp, sg, d2, mka))
    st_o, pr_o, vt_o, sig_o = (t.ap() for t in (st_o, pr_o, vt_o, sig_o))

    with tile.TileContext(nc) as tc:
        with (
            tc.tile_pool(name="const", bufs=1) as constp,
            tc.tile_pool(name="io", bufs=3) as iop,
            tc.tile_pool(name="sigp", bufs=1) as sigp,
            tc.tile_pool(name="et", bufs=4) as etp,
            tc.tile_pool(name="sm", bufs=3) as smp,
            tc.tile_pool(name="st", bufs=4) as stp,
            tc.tile_pool(name="pri", bufs=6) as prip,
            tc.tile_pool(name="ps_s", bufs=2, space="PSUM") as ps_s,
            tc.tile_pool(name="ps_v", bufs=2, space="PSUM") as ps_v,
        ):
            d2_t = constp.tile([128, NCH * PRIOR_W], f32, tag="d2")
            mka_t = constp.tile([128, 128], bf16, tag="mka")
            nc.sync.dma_start(mka_t[:], mka)
            b3_t = constp.tile([128, 1], f32, tag="b3")
            nc.vector.memset(b3_t[:], LN3 * 1e-5)

            # ---- sigma transform for all slices up-front (batches the
            # Exp/Ln activations so the ACT table set switches at most a
            # couple of times before the main all-Exp phase). ----
            sig_coef = []
            for g in range(G_PER_CORE):
                srw = sigp.tile([128, NCH], f32, tag=f"sraw{g}")
                nc.scalar.dma_start(srw[:], sg[g])
                u = sigp.tile([128, NCH], f32, tag=f"u{g}")
                nc.scalar.activation(u[:], srw[:], Act.Exp, scale=-5.0)
                w = sigp.tile([128, NCH], f32, tag=f"w{g}")
                nc.vector.tensor_scalar_add(w[:], u[:], 1.0)
                s0 = sigp.tile([128, NCH], f32, tag=f"s0{g}")
                nc.vector.reciprocal(s0[:], w[:])  # sigmoid(5x)
                sigv = sigp.tile([128, NCH], f32, tag=f"sv{g}")
                nc.scalar.activation(sigv[:], s0[:], Act.Exp, scale=LN3,
                                     bias=b3_t[:])  # 3**(s0+1e-5)
                nc.vector.tensor_scalar_add(sigv[:], sigv[:], -1.0)
                s2 = sigp.tile([128, NCH], f32, tag=f"s2{g}")
                nc.vector.tensor_mul(s2[:], sigv[:], sigv[:])
                r2 = sigp.tile([128, NCH], f32, tag=f"r2{g}")
                nc.vector.reciprocal(r2[:], s2[:])
                a_t = sigp.tile([128, NCH], f32, tag=f"a{g}")
                nc.vector.tensor_scalar_mul(a_t[:], r2[:], -0.5)
                sig_coef.append((sigv, a_t))
            lb_ts = []
            for g in range(G_PER_CORE):
                sigv, _ = sig_coef[g]
                lb = sigp.tile([128, NCH], f32, tag=f"lb{g}")
                # ln(sig/c) then negate -> ln(c/sig)
                nc.scalar.activation(lb[:], sigv[:], Act.Ln,
                                     scale=1.0 / INV_SQRT_2PI)
                nc.vector.tensor_scalar_mul(lb[:], lb[:], -1.0)
                lb_ts.append(lb)

            def emit_prior(g, i):
                _, a_t = sig_coef[g]
                lb_t = lb_ts[g]
                pt = prip.tile([128, PRIOR_W], bf16, tag="pt")
                nc.scalar.activation(
                    pt[:], d2_t[:, i * PRIOR_W:(i + 1) * PRIOR_W],
                    Act.Exp, scale=a_t[:, i:i + 1], bias=lb_t[:, i:i + 1])
                eng = nc.sync if i % 2 == 0 else nc.gpsimd
                eng.dma_start(
                    pr_o[g, i * 128:(i + 1) * 128,
                         PRIOR_S0[i]:PRIOR_S0[i] + PRIOR_W], pt[:])

            def load_inputs(g):
                qt_t = iop.tile([E, L], mm_dt, tag="qt")
                nc.sync.dma_start(qt_t[:], qT[g])
                kt_t = iop.tile([E, L], mm_dt, tag="kt")
                nc.sync.dma_start(kt_t[:], kT[g])
                vp_t = iop.tile([128, NCH * (E + 1)], bf16, tag="vp")
                nc.sync.dma_start(
                    vp_t[:].rearrange("p (c w) -> p c w", w=E + 1),
                    vp[g].rearrange("(c p) w -> p c w", p=128),
                )
                return qt_t, kt_t, vp_t

            nxt_inputs = load_inputs(0)
            for g in range(G_PER_CORE):
                qt_t, kt_t, vp_t = nxt_inputs

                # vt accumulator [E+1, L]: row E collects softmax row-sums.
                vtp = ps_v.tile([E + 1, L], f32, tag="vt")

                # Group chunks into <=1024-wide (2-bank) PSUM tiles so each
                # group needs a single exp activation while leaving room to
                # double-buffer the vt accumulator.
                GROUPS = [(0,), (1, 7), (2, 6), (3, 5), (4,)]
                ets = {}
                pr_q = list(range(NCH))  # prev slice's prior chunks to emit
                for gi, grp in enumerate(GROUPS):
                    members = []
                    off = 0
                    for j in grp:
                        members.append((j, 128 * j, off, L - 128 * j))
                        off += L - 128 * j
                    width = off
                    et = etp.tile([128, width], bf16, tag=f"et{gi}")
                    ps = ps_s.tile([128, width], f32, tag="ps")
                    if g == 0:
                        # just-in-time d2 chunks; a single up-front 4MB DMA
                        # stalls the whole pipeline start for ~14us.
                        for i in range(gi * 2, min(gi * 2 + 2, NCH)):
                            nc.gpsimd.dma_start(
                                d2_t[:, i * PRIOR_W:(i + 1) * PRIOR_W],
                                d2[:, i * PRIOR_W:(i + 1) * PRIOR_W])
                    for (j, l0, off, w) in members:
                        r0 = off
                        while r0 < off + w:
                            rw = min(512 - r0 % 512, off + w - r0)
                            nc.tensor.matmul(
                                ps[:, r0:r0 + rw],
                                kt_t[:, l0:l0 + 128],
                                qt_t[:, l0 + (r0 - off):l0 + (r0 - off) + rw],
                                start=True, stop=True,
                            )
                            r0 += rw
                    nc.scalar.activation(et[:], ps[:], Act.Exp, scale=SCALE)
                    for (j, l0, off, w) in members:
                        # causal mask on the diag block (keep s <= l)
                        nc.vector.tensor_mul(et[:, off:off + 128],
                                             et[:, off:off + 128], mka_t[:])
                    if g > 0:
                        for _ in range(2):
                            if pr_q:
                                emit_prior(g - 1, pr_q.pop(0))
                    for (j, l0, off, w) in members:
                        for (c0, cw) in _chunks(l0):
                            nc.tensor.matmul(
                                vtp[:, c0:c0 + cw],
                                vp_t[:, j * (E + 1):(j + 1) * (E + 1)],
                                et[:, off + c0 - l0:off + c0 - l0 + cw],
                                start=(j == 0), stop=(j == NCH - 1),
                                skip_group_check=True,
                            )
                        ets[j] = (et, off)

                if g + 1 < G_PER_CORE:
                    nxt_inputs = load_inputs(g + 1)

                # 1/rowsum (fast approx is ~3e-6 rel err, plenty here),
                # then broadcast down all 128 partitions on the idle GpSimd.
                rs = smp.tile([1, L], f32, tag="rs")
                nc.scalar.copy(rs[:], vtp[E:E + 1, :])
                rr = smp.tile([1, L], f32, tag="rr")
                nc.vector.reciprocal_approx_fast(rr[:], rs[:])
                bc_s = smp.tile([128, L], f32, tag="bc_s")
                nc.gpsimd.partition_broadcast(bc_s[:], rr[:])
                bc_b = smp.tile([128, L], bf16, tag="bc_b")
                nc.vector.tensor_copy(bc_b[:], bc_s[:])

                # normalize V^T and store
                vn = smp.tile([E, L], f32, tag="vn")
                nc.vector.tensor_mul(vn[:], vtp[:E, :], bc_s[:E, :])
                nc.sync.dma_start(vt_o[g], vn[:])

                # normalize series^T in place and store; the strictly
                # lower-triangular remainder of st_o stays pre-zeroed.
                for j in range(NCH):
                    l0 = 128 * j
                    et, off = ets[j]
                    st_t = stp.tile([128, L - l0], bf16, tag=f"st{j}")
                    nc.vector.tensor_mul(
                        st_t[:], et[:, off:off + L - l0], bc_b[:, l0:])
                    nc.sync.dma_start(st_o[g, l0:l0 + 128, l0:], st_t[:])

            # final slice's priors overlap the last epilogue
            for i in range(NCH):
                emit_prior(G_PER_CORE - 1, i)

            # tiny sigma outputs last so they never gate the input queue
            for g in range(G_PER_CORE):
                nc.sync.dma_start(sig_o[g], sig_coef[g][0][:])

    nc.compile()
    return nc


def _host_inputs(queries, keys, values, sigma):
    q = np.ascontiguousarray(
        queries.transpose(0, 2, 3, 1)).reshape(B * H, E, L)
    k = np.ascontiguousarray(keys.transpose(0, 2, 3, 1)).reshape(B * H, E, L)
    v = np.ascontiguousarray(
        values.transpose(0, 2, 1, 3)).reshape(B * H, L, E)
    vp = np.concatenate([v, np.ones((B * H, L, 1), np.float32)],
                        axis=2).astype(ml_dtypes.bfloat16)
    sgt = np.ascontiguousarray(
        sigma.transpose(0, 2, 1).reshape(B * H, NCH, 128).transpose(0, 2, 1))
    p = np.arange(128, dtype=np.float32)
    w = np.arange(PRIOR_W, dtype=np.float32)
    d2 = np.empty((128, NCH, PRIOR_W), np.float32)
    for i in range(NCH):
        d2[:, i, :] = (128 * i + p[:, None] - (PRIOR_S0[i] + w[None, :])) ** 2
    d2 = np.ascontiguousarray(d2.reshape(128, NCH * PRIOR_W))
    mka = np.where(np.arange(128)[:, None] <= np.arange(128)[None, :],
                   1.0, 0.0).astype(ml_dtypes.bfloat16)
    in_maps = []
    for c in range(N_CORES):
        sl = slice(G_PER_CORE * c, G_PER_CORE * (c + 1))
        in_maps.append({
            "qt": np.ascontiguousarray(q[sl]),
            "kt": np.ascontiguousarray(k[sl]),
            "vp": np.ascontiguousarray(vp[sl]),
            "sg": np.ascontiguousarray(sgt[sl]),
            "d2": d2, "mka": mka,
        })
    return in_maps


LAST_EXEC_NS = None


def kernel(queries, keys, values, sigma):
    global LAST_EXEC_NS
    import concourse.bass_utils as bass_utils

    queries = np.asarray(queries, dtype=np.float32)
    keys = np.asarray(keys, dtype=np.float32)
    values = np.asarray(values, dtype=np.float32)
    sigma = np.asarray(sigma, dtype=np.float32)

    if "nc" not in _CACHE:
        _CACHE["nc"] = _build()
    nc = _CACHE["nc"]
    in_maps = _host_inputs(queries, keys, values, sigma)

    trace = os.environ.get("ANOM_TRACE", "0") == "1"
    kwargs = {}
    if trace:
        import contextlib
        import ctypes
        import types

        if "antenv.axon_hooks" not in sys.modules:
            boot = "/root/.axon_site/trn_agent_boot"
            if boot not in sys.path:
                sys.path.insert(0, boot)
            import trn_boot
            hook = trn_boot._ntff_profile_via_ctypes(
                "/opt/axon/libaxon_pjrt.so")
            mod = types.ModuleType("antenv.axon_hooks")
            mod.get_axon_ntff_profile_hook = lambda: hook
            mod.set_axon_ntff_profile_hook = lambda h: None
            sys.modules["antenv.axon_hooks"] = mod
        bass_utils.upload_artifacts = lambda tmpdir: f"file://{tmpdir}"
        kwargs["trace"] = True

    res = bass_utils.run_bass_kernel_spmd(
        nc, in_maps, core_ids=list(range(N_CORES)), **kwargs)
    LAST_EXEC_NS = res.exec_time_ns

    V = np.empty((B, L, H, E), np.float32)
    series = np.empty((B, H, L, L), np.float32)
    prior = np.empty((B, H, L, L), np.float32)
    sig_small = np.empty((B, H, L), np.float32)
    for c in range(N_CORES):
        r = res.results[c]
        for li in range(G_PER_CORE):
            g = G_PER_CORE * c + li
            b, h = g // H, g % H
            series[b, h] = r["st"][li].T.astype(np.float32)
            prior[b, h] = r["pr"][li].astype(np.float32)
            V[b, :, h, :] = r["vt"][li].T
            sig_small[b, h] = r["sig"][li].T.reshape(L)
    sig = np.broadcast_to(sig_small[..., None], (B, H, L, L))
    return V, series, prior, sig
